# revision 25
# baseline (speedup 1.0000x reference)
"""Trainium2 Bass kernel for a 2-layer GRU teacher-forced decoder.

Math (per reference):
  toks[t,b]: t=0 -> SOS(=1), t>=1 -> target[b, t]   (T = ML-1 = 63 steps)
  x_t = relu(emb[toks[t]])                          [B, E]
  h0 <- GRUCell(x_t, h0; W_ih0, W_hh0, b_ih0, b_hh0)
  h1 <- GRUCell(h0, h1; W_ih1, W_hh1, b_ih1, b_hh1)
  logits_t = h1 @ W_out.T + b_out                   [B, V]
  out = stack(logits).transpose(1,0,2)              [B, T, V]

Device strategy (8 cores, SPMD, no collectives):
  - The sequential GRU recurrence is replicated on every core; the large
    output projection [T*B, H] @ [H, V] is sharded column-wise (vocab)
    8 ways: core k computes logits[:, k*4000:(k+1)*4000].
  - All matmuls run in bf16; gate elementwise math and state are fp32.

Performance structure (v2):
  - Gate matmuls are emitted k-outer / q-inner so consecutive matmuls hit
    different PE column groups (tile_position (0,32q)) and stream
    concurrently through the 4 column-group XBUSes.
  - L0 and L1 are software-pipelined with a 1-step skew: iteration t runs
    L0 step t and L1 step t-1, ordering PE work so the Vector/Scalar
    elementwise chains always overlap matmul streams of the other layer.
    This keeps the PE dense, which keeps the HAM clock gate at 8/8
    (2.4 GHz); a per-step idle gap would re-throttle it to 1.2 GHz.
  - h1T states stream to an HBM history buffer each step; the logits GEMM
    runs as a dense end phase (vocab-sharded), with W_out chunk DMAs
    overlapped with the matmul stream.

Layouts:
  - Gate tensors live in PSUM as [128, 1024] = [(quarter q, batch b),
    (region, j)] where hidden = q*256 + j; regions are rz (512), n_rec
    (256), n_in (256). The 4 hidden-quarters map to the 4 PE column
    groups via tile_position=(0, 32q).
  - State h is kept twice: fp32 "quarter layout" [128, 256] for
    elementwise, and transposed bf16 [128(hidden chunk), 32(batch)] ring
    slots for use as the next matmul's lhsT.
  - PE matmuls on this stack can only read the stationary operand from SBUF
    base partition 0, so the state transpose stages h' quarters into a flat
    [32, 1024] tile via SBUF->SBUF DMA before the PE transposes.
  - The output-projection bias is added on the host after gathering the
    vocab shards.
"""

import os
import sys
import numpy as np

sys.path.insert(0, "/opt/trn_rl_repo")

import ml_dtypes

V, E, H, B, ML = 32000, 512, 1024, 32, 64
SOS = 1
T = ML - 1          # 63
TB = T * B          # 2016
NCORES = 8
VS = V // NCORES    # 4000 vocab slice per core
Q = 4               # hidden quarters
J = H // Q          # 256
KH = H // 128       # 8 contraction chunks over H
KE = E // 128       # 4 contraction chunks over E
MT = 126            # logits M-tile (2016 = 16 * 126)
NMT = TB // MT      # 16
NS = 500            # logits psum slice width (one 2KB fp32 bank)

_BF = ml_dtypes.bfloat16


def _bf16(x):
    return np.asarray(x, np.float32).astype(_BF)


def _prep_wT(w, kchunks):
    """w: [3H, K*128] fp32 -> [128, kchunks, 3H] bf16 with [p, k, col] = w[col, 128k+p]."""
    wt = np.ascontiguousarray(np.asarray(w, np.float32).T)       # [K, 3H]
    wt = wt.reshape(kchunks, 128, wt.shape[1]).transpose(1, 0, 2)  # [128, k, 3H]
    return _bf16(wt)


def _prep_hq(h):
    """h: [B, H] fp32 -> quarter layout [128, 256], [32q+b, j] = h[b, q*256+j]."""
    hq = np.asarray(h, np.float32).reshape(B, Q, J).transpose(1, 0, 2).reshape(Q * B, J)
    return np.ascontiguousarray(hq)


def _prep_hT(h):
    """h: [B, H] -> [128, 8, 32] bf16 with [p, k, b] = h[b, 128k+p]."""
    ht = np.asarray(h, np.float32).T.reshape(KH, 128, B).transpose(1, 0, 2)
    return _bf16(ht)


# Permuted hidden layout used for every tensor whose contraction is over H.
# Position (p=32q+i, c) holds hidden unit 256q + 32c + i, which is exactly
# what the DVE 32x32 block transpose produces from the quarter-layout state
# [32q+b, 256q-relative j=32c+i]: out[32q+i, (c,b)] = h[b, 256q+32c+i].

def _perm_rows(m):
    """m: [H, cols] -> [128, KH, cols] in the permuted-hidden layout."""
    cols = m.shape[1]
    return np.ascontiguousarray(
        m.reshape(Q, KH, 32, cols).transpose(0, 2, 1, 3).reshape(128, KH, cols))


def _prep_wT_perm(w):
    """w: [3H, H] fp32 -> [128, KH, 3H] bf16, permuted-hidden contraction."""
    wt = np.ascontiguousarray(np.asarray(w, np.float32).T)       # [H, 3H]
    return _bf16(_perm_rows(wt))


def _prep_hT_perm(h):
    """h: [B, H] -> [128, KH, B] bf16, permuted-hidden layout."""
    ht = np.ascontiguousarray(np.asarray(h, np.float32).T)       # [H, B]
    return _bf16(_perm_rows(ht))


def _gate_bias_quarter(b_ih, b_hh):
    """Quarter-layout fp32 bias tiles for the DVE adds.

    bq [128, 768]: [32q+b, gate*256+j] = (b_ih+b_hh) for r,z; b_hh for n.
    bc [128, 256]: [32q+b, j] = b_ih n-part.
    """
    bi = np.asarray(b_ih, np.float32)
    bh = np.asarray(b_hh, np.float32)
    comb = np.empty(3 * H, np.float32)
    comb[0:2 * H] = bi[0:2 * H] + bh[0:2 * H]
    comb[2 * H:] = bh[2 * H:]
    bq = np.empty((128, 3 * J), np.float32)
    bc = np.empty((128, J), np.float32)
    for q in range(Q):
        s = q * J
        row = np.concatenate([comb[s:s + J], comb[H + s:H + s + J],
                              comb[2 * H + s:2 * H + s + J]])
        bq[32 * q:32 * (q + 1)] = row[None, :]
        bc[32 * q:32 * (q + 1)] = bi[2 * H + s:2 * H + s + J][None, :]
    return bq, bc


def _build_inputs(encoder_hidden, target_tensor, emb,
                  W_ih0, W_hh0, b_ih0, b_hh0, W_ih1, W_hh1, b_ih1, b_hh1,
                  W_out, b_out):
    """Host-side layout prep. Returns (shared_map, per_core_maps)."""
    tt = np.asarray(target_tensor)
    toks = np.concatenate(
        [np.full((B, 1), SOS, dtype=tt.dtype), tt[:, 1:ML - 1]], axis=1).T  # [T, B]
    X = np.maximum(np.asarray(emb, np.float32)[toks], 0.0)  # [T, B, E]
    # xT [128, KE, T*B]: [p, k, t*32+b] = X[t, b, 128k+p]
    xT = X.reshape(TB, KE, 128).transpose(2, 1, 0)
    xT = np.ascontiguousarray(_bf16(xT))

    bq0, bc0 = _gate_bias_quarter(b_ih0, b_hh0)
    bq1, bc1 = _gate_bias_quarter(b_ih1, b_hh1)

    shared = {
        "xT": xT,
        "h0q": _prep_hq(encoder_hidden[0]),
        "h1q": _prep_hq(encoder_hidden[1]),
        "h0T": _prep_hT_perm(encoder_hidden[0]),
        "h1T": _prep_hT_perm(encoder_hidden[1]),
        "wih0T": _prep_wT(W_ih0, KE),
        "whh0T": _prep_wT_perm(W_hh0),
        "wih1T": _prep_wT_perm(W_ih1),
        "whh1T": _prep_wT_perm(W_hh1),
        "bq0": bq0, "bc0": bc0,
        "bq1": bq1, "bc1": bc1,
    }
    wout = np.asarray(W_out, np.float32)
    per_core = []
    for c in range(NCORES):
        sl = slice(c * VS, (c + 1) * VS)
        woutT = _perm_rows(np.ascontiguousarray(wout[sl].T))  # [128, 8, VS]
        per_core.append({
            "woutT": np.ascontiguousarray(_bf16(woutT)),
        })
    return shared, per_core


# ---------------------------------------------------------------------------
# Device program
# ---------------------------------------------------------------------------

def _emit(nc, tc, io, n_steps=T):
    import concourse.bass as bass
    from concourse import mybir
    from concourse.alu_op_type import AluOpType as alu

    f32 = mybir.dt.float32
    bf16 = mybir.dt.bfloat16
    Sig = mybir.ActivationFunctionType.Sigmoid
    Tanh = mybir.ActivationFunctionType.Tanh

    NRING = 4          # state ring slots (h0T / h1T)

    ctx_pools = []

    def pool(name, bufs, space="SBUF"):
        p = tc.tile_pool(name=name, bufs=bufs, space=space)
        ctx_pools.append(p)
        return p.__enter__()

    consts = pool("consts", 1)
    state = pool("state", 1)
    hqp = pool("hq", 2)
    work = pool("work", 1)
    xp = pool("xs", 3)

    # ---- constants / persistent tensors in SBUF ----
    bq = {}
    bc = {}
    for L in (0, 1):
        bq[L] = consts.tile([128, 3 * J], f32, tag=f"bq{L}", name=f"bq{L}")
        nc.sync.dma_start(bq[L][:], io[f"bq{L}"][:])
        bc[L] = consts.tile([128, J], f32, tag=f"bc{L}", name=f"bc{L}")
        nc.sync.dma_start(bc[L][:], io[f"bc{L}"][:])

    # state rings, transposed bf16, permuted-hidden: [128, slot, kchunk, 32]
    HT = {}
    for L in (0, 1):
        HT[L] = state.tile([128, NRING, KH, 32], bf16, tag=f"H{L}T", name=f"H{L}T")
        # initial state h(-1) lives in ring slot NRING-1 (= -1 mod NRING)
        nc.sync.dma_start(HT[L][:, NRING - 1], io[f"h{L}T"][:])

    hq_init = {}
    for L in (0, 1):
        hq_init[L] = consts.tile([128, J], f32, tag=f"hq{L}i", name=f"hq{L}i")
        nc.sync.dma_start(hq_init[L][:], io[f"h{L}q"][:])

    def slot(L, t):
        return HT[L][:, t % NRING]

    # ---- gate matmul emitters (k-outer / q-inner for col-group overlap) ----

    def mms_inp_first(G, Wa, kofs, kw, lhsT_of):
        """Input-path MMs when they run BEFORE the recurrent MMs (layer 0).

        rz region: start only; C region: complete group."""
        for k in range(kw):
            lhsT = lhsT_of(k)
            w3 = Wa[:, kofs + k, :].rearrange("p (g j) -> p g j", g=3)
            for q in range(Q):
                Gq_rz = G[32 * q:32 * q + 32, 0:2 * J]
                nc.tensor.matmul(Gq_rz.rearrange("p (g j) -> p g j", g=2),
                                 lhsT, w3[:, 0:2, q * J:(q + 1) * J],
                                 start=(k == 0), stop=False,
                                 tile_position=(0, 32 * q), skip_group_check=True)
            for q in range(Q):
                Cq = G[32 * q:32 * q + 32, 3 * J:4 * J]
                nc.tensor.matmul(Cq, lhsT, w3[:, 2, q * J:(q + 1) * J],
                                 start=(k == 0), stop=(k == kw - 1),
                                 tile_position=(0, 32 * q), skip_group_check=True)

    def mms_rec_last(G, Wa, kofs, lhsT_of):
        """Recurrent-path MMs when they run AFTER the input MMs (layer 0)."""
        for k in range(KH):
            lhsT = lhsT_of(k)
            w3 = Wa[:, kofs + k, :].rearrange("p (g j) -> p g j", g=3)
            for q in range(Q):
                Gq_rz = G[32 * q:32 * q + 32, 0:2 * J]
                nc.tensor.matmul(Gq_rz.rearrange("p (g j) -> p g j", g=2),
                                 lhsT, w3[:, 0:2, q * J:(q + 1) * J],
                                 start=False, stop=(k == KH - 1),
                                 tile_position=(0, 32 * q), skip_group_check=True)
            for q in range(Q):
                Gq_n = G[32 * q:32 * q + 32, 2 * J:3 * J]
                nc.tensor.matmul(Gq_n, lhsT, w3[:, 2, q * J:(q + 1) * J],
                                 start=(k == 0), stop=(k == KH - 1),
                                 tile_position=(0, 32 * q), skip_group_check=True)

    def mms_rec_first(G, Wa, kofs, lhsT_of):
        """Recurrent-path MMs when they run BEFORE the input MMs (layer 1)."""
        for k in range(KH):
            lhsT = lhsT_of(k)
            w3 = Wa[:, kofs + k, :].rearrange("p (g j) -> p g j", g=3)
            for q in range(Q):
                Gq_rz = G[32 * q:32 * q + 32, 0:2 * J]
                nc.tensor.matmul(Gq_rz.rearrange("p (g j) -> p g j", g=2),
                                 lhsT, w3[:, 0:2, q * J:(q + 1) * J],
                                 start=(k == 0), stop=False,
                                 tile_position=(0, 32 * q), skip_group_check=True)
            for q in range(Q):
                Gq_n = G[32 * q:32 * q + 32, 2 * J:3 * J]
                nc.tensor.matmul(Gq_n, lhsT, w3[:, 2, q * J:(q + 1) * J],
                                 start=(k == 0), stop=(k == KH - 1),
                                 tile_position=(0, 32 * q), skip_group_check=True)

    def mms_inp_last(G, Wa, kofs, kw, lhsT_of):
        """Input-path MMs when they run AFTER the recurrent MMs (layer 1)."""
        for k in range(kw):
            lhsT = lhsT_of(k)
            w3 = Wa[:, kofs + k, :].rearrange("p (g j) -> p g j", g=3)
            for q in range(Q):
                Gq_rz = G[32 * q:32 * q + 32, 0:2 * J]
                nc.tensor.matmul(Gq_rz.rearrange("p (g j) -> p g j", g=2),
                                 lhsT, w3[:, 0:2, q * J:(q + 1) * J],
                                 start=False, stop=(k == kw - 1),
                                 tile_position=(0, 32 * q), skip_group_check=True)
            for q in range(Q):
                Cq = G[32 * q:32 * q + 32, 3 * J:4 * J]
                nc.tensor.matmul(Cq, lhsT, w3[:, 2, q * J:(q + 1) * J],
                                 start=(k == 0), stop=(k == kw - 1),
                                 tile_position=(0, 32 * q), skip_group_check=True)

    def gate_elem(G, hq_prev, layer):
        """sigmoid/tanh + gated update on Vector/Scalar; returns hq_new."""
        Sp = work.tile([128, 2 * J], f32, tag=f"Sp{layer}")
        nc.vector.tensor_tensor(Sp[:], G[:, 0:2 * J], bq[layer][:, 0:2 * J],
                                alu.add)
        nc.scalar.activation(Sp[:, 0:J], Sp[:, 0:J], Sig)          # r (in place)
        nc.scalar.activation(Sp[:, J:2 * J], Sp[:, J:2 * J], Sig)  # z (in place)
        t0 = work.tile([128, J], f32, tag=f"t0{layer}")
        nc.vector.tensor_tensor(t0[:], G[:, 2 * J:3 * J], bq[layer][:, 2 * J:3 * J],
                                alu.add)
        nc.vector.tensor_tensor(t0[:], Sp[:, 0:J], t0[:], alu.mult)   # r*gh_n
        t2 = work.tile([128, J], f32, tag=f"t2{layer}")
        nc.vector.tensor_tensor(t2[:], G[:, 3 * J:4 * J], bc[layer][:], alu.add)
        nc.vector.tensor_tensor(t2[:], t2[:], t0[:], alu.add)
        n_t = work.tile([128, J], f32, tag=f"n{layer}")
        nc.scalar.activation(n_t[:], t2[:], Tanh)
        # SBUF-only tail runs on GpSimd to keep the Vector queue short.
        # 1-z overwrites the (now dead) r region of Sp
        nc.gpsimd.tensor_scalar(Sp[:, 0:J], Sp[:, J:2 * J], -1.0, 1.0,
                                alu.mult, alu.add)
        zh = work.tile([128, J], f32, tag=f"zh{layer}")
        nc.gpsimd.tensor_tensor(zh[:], Sp[:, J:2 * J], hq_prev[:], alu.mult)
        nc.gpsimd.tensor_tensor(n_t[:], n_t[:], Sp[:, 0:J], alu.mult)  # (1-z)*n
        hq_new = hqp.tile([128, J], f32, tag=f"hq{layer}")
        nc.vector.tensor_tensor(hq_new[:], n_t[:], zh[:], alu.add)
        # bf16 twin of h' produced in parallel on GpSimd (not a serial copy)
        hb = work.tile([128, J], bf16, tag=f"hb{layer}")
        nc.gpsimd.tensor_tensor(hb[:], n_t[:], zh[:], alu.add)
        return hq_new, hb

    def transpose_state(hb, layer, t, to_hbm=False):
        """DVE 32x32 block transpose: quarter-layout bf16 h' -> permuted-
        hidden lhsT [128, KH, 32] in HT ring slot t (see _perm_rows)."""
        dst = HT[layer][:, t % NRING]
        nc.vector.transpose(dst.rearrange("p k b -> p (k b)"), hb[:])
        if to_hbm:
            nc.sync.dma_start(io["h1hist"][:, :, t * 32:(t + 1) * 32], dst)

    def load_x(t):
        xt = xp.tile([128, KE, 32], bf16, tag="xt")
        nc.sync.dma_start(xt[:], io["xT"][:, :, t * 32:(t + 1) * 32])
        return xt

    # =================== merged pipelined recurrence ===================
    with tc.tile_pool(name="arena", bufs=1) as arena_p, \
         tc.tile_pool(name="psumG0", bufs=2, space="PSUM") as psumG0, \
         tc.tile_pool(name="psumG1", bufs=2, space="PSUM") as psumG1:

        # weight arena: [128, 4+8+8+8, 3H] bf16, chunk-granular DMAs so the
        # first matmuls can start as soon as their k-chunk lands. Load
        # order follows first use: wih0T (prologue input MMs), whh0T (A),
        # then the layer-1 weights.
        WIH0, WHH0, WIH1, WHH1 = 0, KE, KE + KH, KE + 2 * KH
        a = arena_p.tile([128, KE + 3 * KH, 3 * H], bf16, tag="arena", name="arena")
        xs = {0: load_x(0), 1: load_x(1)}
        for k in range(KE):
            nc.sync.dma_start(a[:, WIH0 + k, :], io["wih0T"][:, k, :])
        for k in range(KH):
            nc.sync.dma_start(a[:, WHH0 + k, :], io["whh0T"][:, k, :])
        for k in range(KH):
            nc.sync.dma_start(a[:, WHH1 + k, :], io["whh1T"][:, k, :])
        for k in range(KH):
            nc.sync.dma_start(a[:, WIH1 + k, :], io["wih1T"][:, k, :])
        hq0_prev = hq_init[0]
        hq1_prev = hq_init[1]

        # prologue: G0(0) input path
        G0 = {0: psumG0.tile([128, 4 * J], f32, tag="G0", name="G0")}
        mms_inp_first(G0[0], a, WIH0, KE, lambda k, x=xs[0]: x[:, k, :])

        for t in range(n_steps):
            # --- A: L0 recurrent MMs, completing G0(t) ---
            mms_rec_last(G0[t], a, WHH0,
                         lambda k, tt=t: slot(0, tt - 1)[:, k])
            # --- L0 elementwise (V/S/GP) + DVE transpose, overlapped with C/D ---
            hq0_prev, hb0 = gate_elem(G0[t], hq0_prev, 0)
            transpose_state(hb0, 0, t)
            del G0[t]

            # --- C+D: L1 step t-1: recurrent then input-path MMs ---
            if t >= 1:
                G1 = psumG1.tile([128, 4 * J], f32, tag="G1", name="G1")
                mms_rec_first(G1, a, WHH1,
                              lambda k, tt=t - 1: slot(1, tt - 1)[:, k])
                mms_inp_last(G1, a, WIH1, KH,
                             lambda k, tt=t - 1: slot(0, tt)[:, k])
                hq1_prev, hb1 = gate_elem(G1, hq1_prev, 1)
                transpose_state(hb1, 1, t - 1, to_hbm=True)

            # --- F: L0 input MMs for step t+1 ---
            if t + 1 < n_steps:
                if t + 2 < n_steps:
                    xs[t + 2] = load_x(t + 2)
                G0[t + 1] = psumG0.tile([128, 4 * J], f32, tag="G0", name="G0")
                mms_inp_first(G0[t + 1], a, WIH0, KE,
                              lambda k, x=xs[t + 1]: x[:, k, :])
                del xs[t + 1]

        # epilogue: L1 step n_steps-1
        G1 = psumG1.tile([128, 4 * J], f32, tag="G1", name="G1")
        mms_rec_first(G1, a, WHH1,
                      lambda k: slot(1, n_steps - 2)[:, k])
        mms_inp_last(G1, a, WIH1, KH,
                     lambda k: slot(0, n_steps - 1)[:, k])
        hq1_prev, hb1 = gate_elem(G1, hq1_prev, 1)
        transpose_state(hb1, 1, n_steps - 1, to_hbm=True)

    # ================= logits GEMM (vocab-sharded) =================
    with tc.tile_pool(name="psumL", bufs=4, space="PSUM") as psumL, \
         tc.tile_pool(name="loadp", bufs=1) as loadp, \
         tc.tile_pool(name="outp", bufs=3) as outp:
        # load order matches first use: wout slice 0 + h1 m-tile 0 first
        h1full = loadp.tile([128, KH, TB], bf16, tag="h1f", name="h1f")
        wout = loadp.tile([128, KH, VS], bf16, tag="wout", name="wout")
        nc.sync.dma_start(wout[:, :, 0:NS], io["woutT"][:, :, 0:NS])
        for m in range(NMT):
            nc.sync.dma_start(h1full[:, :, m * MT:(m + 1) * MT],
                              io["h1hist"][:, :, m * MT:(m + 1) * MT])
        for s in range(1, VS // NS):
            nc.sync.dma_start(wout[:, :, s * NS:(s + 1) * NS],
                              io["woutT"][:, :, s * NS:(s + 1) * NS])
        for s in range(VS // NS):
            for m in range(NMT):
                Lt = psumL.tile([128, NS], f32, tag="L", name="L")
                for k in range(KH):
                    nc.tensor.matmul(
                        Lt[0:MT, :],
                        h1full[:, k, m * MT:(m + 1) * MT],
                        wout[:, k, s * NS:(s + 1) * NS],
                        start=(k == 0), stop=(k == KH - 1))
                ob = outp.tile([128, NS], f32, tag="ob", name="ob")
                nc.vector.tensor_copy(ob[0:MT, :], Lt[0:MT, :])
                nc.sync.dma_start(
                    io["logits"][m * MT:(m + 1) * MT, s * NS:(s + 1) * NS],
                    ob[0:MT, :])

    for p in reversed(ctx_pools):
        p.__exit__(None, None, None)


def _build_program(n_steps=T):
    import concourse.bacc as bacc
    import concourse.tile as tile
    from concourse import mybir

    f32 = mybir.dt.float32
    bf16 = mybir.dt.bfloat16

    nc = bacc.Bacc("TRN2", target_bir_lowering=False, debug=False,
                   num_devices=NCORES)

    def din(name, shape, dt):
        return nc.dram_tensor(name, list(shape), dt, kind="ExternalInput").ap()

    io = {
        "xT": din("xT", (128, KE, TB), bf16),
        "h0q": din("h0q", (128, J), f32),
        "h1q": din("h1q", (128, J), f32),
        "h0T": din("h0T", (128, KH, 32), bf16),
        "h1T": din("h1T", (128, KH, 32), bf16),
        "wih0T": din("wih0T", (128, KE, 3 * H), bf16),
        "whh0T": din("whh0T", (128, KH, 3 * H), bf16),
        "wih1T": din("wih1T", (128, KH, 3 * H), bf16),
        "whh1T": din("whh1T", (128, KH, 3 * H), bf16),
        "bq0": din("bq0", (128, 3 * J), f32),
        "bc0": din("bc0", (128, J), f32),
        "bq1": din("bq1", (128, 3 * J), f32),
        "bc1": din("bc1", (128, J), f32),
        "woutT": din("woutT", (128, KH, VS), bf16),
        "h1hist": nc.dram_tensor("h1hist", [128, KH, TB], bf16,
                                 kind="Internal").ap(),
        "logits": nc.dram_tensor("logits", [TB, VS], f32,
                                 kind="ExternalOutput").ap(),
    }

    with tile.TileContext(nc) as tc:
        _emit(nc, tc, io, n_steps=n_steps)

    nc.compile()
    return nc


_CACHED = {}


def _get_program(n_steps=T):
    if n_steps not in _CACHED:
        _CACHED[n_steps] = _build_program(n_steps)
    return _CACHED[n_steps]


def kernel(encoder_outputs, encoder_hidden, target_tensor, emb,
           W_ih0, W_hh0, b_ih0, b_hh0, W_ih1, W_hh1, b_ih1, b_hh1,
           W_out, b_out, _trace=False):
    from concourse import bass_utils

    shared, per_core = _build_inputs(
        encoder_hidden, target_tensor, emb,
        W_ih0, W_hh0, b_ih0, b_hh0, W_ih1, W_hh1, b_ih1, b_hh1, W_out, b_out)

    nc = _get_program()
    in_maps = []
    for c in range(NCORES):
        m = dict(shared)
        m.update(per_core[c])
        in_maps.append(m)

    res = None
    for attempt in range(3):
        try:
            res = bass_utils.run_bass_kernel_spmd(
                nc, in_maps, core_ids=list(range(NCORES)), trace=_trace)
            break
        except Exception:
            if attempt == 2:
                raise
            import time
            time.sleep(20)

    parts = [res.results[c]["logits"].reshape(T, B, VS) for c in range(NCORES)]
    full = np.concatenate(parts, axis=2)          # [T, B, V]
    full += np.asarray(b_out, np.float32)[None, None, :]
    out = np.ascontiguousarray(full.transpose(1, 0, 2)).astype(np.float32)
    if _trace:
        kernel.last_results = res
    return out


kernel.last_results = None
